# revision 13
# baseline (speedup 1.0000x reference)
"""MatchNet kernel for 8 Trainium2 NeuronCores.

Math (per batch b):
    keys   = q[b] @ W + bias
    scores = p[b] @ keys^T
    attn   = softmax(scores, axis=-1)
    out[b] = relu(attn @ q[b])

The Dense bias is dropped: softmax over lq is invariant to the per-lp
constant p@b^T it adds to scores, and keys are not used elsewhere.

Sharding: data-parallel over B=16 across 8 cores (2 batches per core).
W is broadcast. p and q are transposed on the host so every on-chip matmul
has its contraction dim on SBUF partitions.

Precision: all matmuls run as a single fp16 pass (fp32 PSUM accumulation).
Numpy sim of this exact scheme on the real seed-0 inputs: rel_err 1.67e-2
(gate 2e-2, deterministic). The previous revision added fp8-e5m2 DoubleRow
residual corrections to the two score-path matmuls (sim 9.4e-3, HW 9.85e-3,
~52us of extra PE time + 5MB more input DMA); restore those if more margin
is ever needed. Output is computed/stored fp16 (sim-identical: 1.674e-2)
and upcast to fp32 on the host, halving output DMA.
    MM1: keysT[h, lq] = sum_hk W[hk, h] * qT[hk, lq]   (fp16)
         keysT -> k16 (fp16) via DVE copy
    MM2: scores[lp, lq] = sum_h pT[h, lp] * k16[h, lq] (fp16)
    softmax over free dim; exp via ACT (bias=-rowmax, accum rowsum),
    exp output stored fp16
    T:   attnT[lq, lp] via PE transpose (fp16)
    MM3: out[lp, h] = sum_lq attnT[lq, lp] * q[lq, h]  (fp16)
    relu(out * (1/rowsum)) via ACT with per-partition scale, fp16 out
(DMA xbar transpose for attnT was tried instead of PE transposes and was
~110us SLOWER end-to-end — keep the PE-transpose path.)
"""

import os
from contextlib import ExitStack

import numpy as np

import concourse.bass as bass
import concourse.mybir as mybir
import concourse.tile as tile
from concourse import bacc
from concourse.bass import ts
from concourse.bass_utils import run_bass_kernel_spmd
from concourse.masks import make_identity

B, L, H = 16, 1024, 1024
NCORES = 8
BPC = B // NCORES  # batches per core
P = 128
KO = H // P        # 8 contraction chunks
NT = L // P        # 8 lp tiles per batch
NF = 512           # matmul moving free dim
NCH = L // NF      # 2 free chunks
F32 = mybir.dt.float32
F16 = mybir.dt.float16
AF = mybir.ActivationFunctionType
AX = mybir.AxisListType


def _build_body(ctx, tc, ins, out):
    nc = tc.nc
    pT16, qT16, qn16, W16 = ins

    const = ctx.enter_context(tc.tile_pool(name="const", bufs=1))
    ps_big = ctx.enter_context(
        tc.tile_pool(name="psbig", bufs=3, space=bass.MemorySpace.PSUM)
    )

    # PE warmup: the first ~9.5us are DMA-bound (bootstrap + first loads) and
    # the PE would sit idle, entering the kernel HAM-throttled at 1.2 GHz.
    # Zero matmuls during that window cost nothing and keep the clock-gate
    # activity window filled so K=8/8 fires soon after the real matmuls start.
    # The warmup tiles live in the long-lived pools (const / ps_big slot 0):
    # a dedicated pool whose context exits would recycle its SBUF address to
    # W16_sb[0], making the first real LDW wait on ALL warmup matmuls and
    # stalling the load descriptor queue ~2us behind pool-exit bookkeeping
    # (measured: first real MM at 16.9us instead of ~9.5us).
    wsb = const.tile([P, P], F16, name="warm_sb")
    nc.gpsimd.memset(wsb[:], 0.0)
    wps = ps_big.tile([P, L], F32, name="warm_ps", tag="ps_big")
    # The A-half (2MB) cannot land before ~14us: aggregate DMA bandwidth at
    # kernel start is only ~300-400 GB/s no matter how many queues carry it.
    # 64 pairs bridge the PE from the first possible matmul (~8us) to that
    # point with zero idle (cold ~107ns/pair until the HAM un-throttles
    # mid-warmup, ~56ns after), so the dense phase starts warm at 2.4GHz.
    # (32 pairs left a 2.4us gap: the HAM window reset and the entire first
    # m-sweep ran at 1.2GHz until 20.3us — measured ~3us slower.)
    for _ in range(64):
        nc.tensor.matmul(wps[:, :P], wsb[:], wsb[:], start=True, stop=True)

    # W tiles, one per k-chunk PAIR: chunk-pair-granular deps let the first
    # matmuls start as soon as pair 0 lands, while keeping the phase-1 b=0
    # stream at 8 descriptors total (4 sync + 4 scalar) — within the
    # framework's 8 rotating DMA semaphores, so no descriptor ever waits on
    # consumer progress during the DMA-bound start. (Per-chunk descriptors
    # measured ~2.5us slower: the 5th+ load on each queue chains to the PE.)
    W16_sb = [const.tile([P, 2, H], F16, name=f"W16_sb_{kp}") for kp in range(KO // 2)]
    ident = const.tile([P, P], F16)
    make_identity(nc, ident[:])

    qT_pool = ctx.enter_context(tc.tile_pool(name="qTp", bufs=1))
    q_pool = ctx.enter_context(tc.tile_pool(name="qp", bufs=1))
    keysT_pool = ctx.enter_context(tc.tile_pool(name="keysTp", bufs=1))
    pT_pool = ctx.enter_context(tc.tile_pool(name="pTp", bufs=3))
    attn_pool = ctx.enter_context(tc.tile_pool(name="attnp", bufs=2))
    attnT_pool = ctx.enter_context(tc.tile_pool(name="attnTp", bufs=2))
    osb_pool = ctx.enter_context(tc.tile_pool(name="osbp", bufs=2))
    stat_pool = ctx.enter_context(tc.tile_pool(name="statp", bufs=8))
    ps_t = ctx.enter_context(
        tc.tile_pool(name="pst", bufs=2, space=bass.MemorySpace.PSUM)
    )

    W16_re = W16.rearrange("(kp two ki) h -> ki kp two h", ki=P, two=2)

    for b in range(BPC):
        # qT tiles first (MM1 needs them). Issue order = consumption order
        # of the in-order PE stream.
        qT16_sb = [
            qT_pool.tile([P, 2, L], F16, name=f"qT16_sb_{b}_{kp}", tag=f"qT16_sb{kp}")
            for kp in range(KO // 2)
        ]
        qT16_re = qT16[b].rearrange("(kp two ki) l -> ki kp two l", ki=P, two=2)
        # b=0 loads run two queues in parallel (W16 on sync, qT16 on scalar —
        # scalar is idle until its first exp at ~45us) so the A-half chunks
        # land ~3us sooner during the DMA-bound kernel start. b=1's qT16 goes
        # on sync: by then scalar is busy with softmax ACTs, and spreading
        # mid-kernel loads over busy engines measured ~11us slower.
        for kp in range(KO // 2):
            if b == 0:
                nc.sync.dma_start(W16_sb[kp][:], W16_re[:, kp, :, :])
                nc.scalar.dma_start(qT16_sb[kp][:], qT16_re[:, kp, :, :])
            else:
                nc.sync.dma_start(qT16_sb[kp][:], qT16_re[:, kp, :, :])

        # ---- phase 1: keysT[h, lq] = (q @ W)^T, fp16.
        k16_sb = keysT_pool.tile([P, KO, L], F16, name=f"k16_{b}", tag="k16")
        # Each m-group is split into half-contraction sub-groups A (chunks
        # 0-3) and B (chunks 4-7), issued A0 A1 A2 B0 A3 B1 ... so the PE has
        # ~21us of A-work gated only on the first half of the 4MB phase-1
        # stream (the kernel start is DMA-bandwidth-bound). Max 3 PSUM tiles
        # live (m..m+2) matches ps_big bufs=3.
        ps_ks = {}

        def phase1_half(m, half):
            if half == 0:
                ps_ks[m] = ps_big.tile([P, L], F32, name=f"ps_k_{b}_{m}",
                                       tag="ps_big")
            ps_k = ps_ks[m]
            for n in range(NCH):
                for k in range(4 * half, 4 * half + 4):
                    nc.tensor.matmul(
                        ps_k[:, ts(n, NF)],
                        W16_sb[k // 2][:, k % 2, ts(m, P)],
                        qT16_sb[k // 2][:, k % 2, ts(n, NF)],
                        start=(k == 0),
                        stop=(k == KO - 1),
                    )
            if half == 1:
                ps_k = ps_ks.pop(m)
                nc.vector.tensor_copy(k16_sb[:, m, :], ps_k[:])

        for step in range(KO + 3):
            if step >= 3:
                phase1_half(step - 3, 1)
            if step < KO:
                phase1_half(step, 0)

        # q natural (fp16, for MM3): issued after phase-1 compute so its DMA
        # queues drain behind the phase-1-critical loads.
        qn_sb = q_pool.tile([P, KO, H], F16, name=f"qn_sb_{b}", tag="qn_sb")
        qre = qn16[b].rearrange("(kp two ki) h -> ki kp two h", ki=P, two=2)
        for kp in range(KO // 2):
            nc.sync.dma_start(qn_sb[:, 2 * kp : 2 * kp + 2, :], qre[:, kp, :, :])

        # ---- phase 2/3: per lp tile, software-pipelined
        pT16_r = pT16[b].rearrange("(ko ki) l -> ki ko l", ki=P)
        scores_ps = {}
        soft = {}

        def stage_scores(i, b=b, pT16_r=pT16_r, k16_sb=k16_sb):
            p16_sb = pT_pool.tile([P, KO, P], F16, name=f"p16_sb_{b}_{i}",
                                  tag="p16_sb")
            nc.sync.dma_start(p16_sb[:], pT16_r[:, :, ts(i, P)])
            ps_s = ps_big.tile([P, L], F32, name=f"ps_s_{b}_{i}", tag="ps_big")
            for n in range(NCH):
                for k in range(KO):
                    nc.tensor.matmul(
                        ps_s[:, ts(n, NF)],
                        p16_sb[:, k, :],
                        k16_sb[:, k, ts(n, NF)],
                        start=(k == 0),
                        stop=(k == KO - 1),
                    )
            scores_ps[i] = ps_s

        def stage_softmax_t(i, b=b):
            ps_s = scores_ps.pop(i)
            negmax = stat_pool.tile([P, 1], F32, name=f"negmax_{b}_{i}", tag="negmax")
            nc.vector.reduce_max(negmax[:], ps_s[:], axis=AX.X, negate=True)
            attn_sb = attn_pool.tile([P, L], F16, name=f"attn_{b}_{i}", tag="attn")
            rowsum = stat_pool.tile([P, 1], F32, name=f"rowsum_{b}_{i}", tag="rowsum")
            nc.scalar.activation(
                attn_sb[:],
                ps_s[:],
                AF.Exp,
                bias=negmax[:],
                accum_out=rowsum[:],
            )
            recip = stat_pool.tile([P, 1], F32, name=f"recip_{b}_{i}", tag="recip")
            nc.vector.reciprocal(recip[:], rowsum[:])

            attnT_sb = attnT_pool.tile([P, L], F16, name=f"attnT_{b}_{i}", tag="attnT")
            for g in range(L // NF):
                ps_tt = ps_t.tile([P, NF], F16, name=f"ps_tt_{b}_{i}_{g}", tag="ps_t")
                for j in range(NF // P):
                    c = g * (NF // P) + j
                    nc.tensor.transpose(
                        ps_tt[:, ts(j, P)], attn_sb[:, ts(c, P)], ident[:]
                    )
                nc.vector.tensor_copy(attnT_sb[:, ts(g, NF)], ps_tt[:])
            soft[i] = (attnT_sb, recip)

        def stage_mm3(i, b=b, qn_sb=qn_sb):
            attnT_sb, recip = soft.pop(i)
            out_sb = osb_pool.tile([P, H], F16, name=f"out_sb_{b}_{i}", tag="out_sb")
            ps_o = ps_big.tile([P, H], F32, name=f"ps_o_{b}_{i}", tag="ps_big")
            # relu+store per n-chunk so the drain of chunk 0 hides under the
            # matmuls of chunk 1 (shrinks the kernel tail). (Draining the
            # last tile in 256-col pieces was measured 0.5us SLOWER — the
            # ACT fixed cost makes two serialized half-relus beat one full.)
            for n in range(NCH):
                for k in range(KO):
                    nc.tensor.matmul(
                        ps_o[:, ts(n, NF)],
                        attnT_sb[:, ts(k, P)],
                        qn_sb[:, k, ts(n, NF)],
                        start=(k == 0),
                        stop=(k == KO - 1),
                    )
                nc.scalar.activation(
                    out_sb[:, ts(n, NF)], ps_o[:, ts(n, NF)], AF.Relu, scale=recip[:]
                )
                nc.sync.dma_start(out[b, ts(i, P), ts(n, NF)], out_sb[:, ts(n, NF)])

        stage_scores(0)
        stage_scores(1)
        for i in range(NT):
            stage_softmax_t(i)
            if i + 2 < NT:
                stage_scores(i + 2)
            stage_mm3(i)


_IN_NAMES = ["pT16", "qT16", "qn16", "W16"]

_CACHED = None


def _get_program():
    global _CACHED
    if _CACHED is not None:
        return _CACHED
    nc = bacc.Bacc(
        "TRN2",
        target_bir_lowering=False,
        debug=False,
        num_devices=NCORES,
    )
    specs = {
        "pT16": ([BPC, H, L], F16),
        "qT16": ([BPC, H, L], F16),
        "qn16": ([BPC, L, H], F16),
        "W16": ([H, H], F16),
    }
    handles = [
        nc.dram_tensor(name, *specs[name], kind="ExternalInput") for name in _IN_NAMES
    ]
    out_h = nc.dram_tensor("out", [BPC, L, H], F16, kind="ExternalOutput")
    with tile.TileContext(nc) as tc:
        with ExitStack() as ctx:
            _build_body(ctx, tc, [h.ap() for h in handles], out_h.ap())
    nc.compile()
    _CACHED = nc
    return nc


def kernel(p, q, W_key, b_key):
    # b_key is mathematically irrelevant: softmax over lq is invariant to the
    # per-lp constant p@b^T it adds to scores, and keys are not used elsewhere.
    del b_key
    p = np.ascontiguousarray(np.asarray(p, dtype=np.float32))
    q = np.ascontiguousarray(np.asarray(q, dtype=np.float32))
    W = np.ascontiguousarray(np.asarray(W_key, dtype=np.float32))

    pT16 = np.ascontiguousarray(p.transpose(0, 2, 1)).astype(np.float16)
    qT16 = np.ascontiguousarray(q.transpose(0, 2, 1)).astype(np.float16)
    qn16 = q.astype(np.float16)
    W16 = W.astype(np.float16)

    full = {"pT16": pT16, "qT16": qT16, "qn16": qn16}

    in_maps = []
    for c in range(NCORES):
        sl = slice(c * BPC, (c + 1) * BPC)
        m = {k: np.ascontiguousarray(v[sl]) for k, v in full.items()}
        m["W16"] = W16
        in_maps.append(m)

    nc = _get_program()
    trace = bool(int(os.environ.get("MATCHNET_TRACE", "0")))
    res = run_bass_kernel_spmd(nc, in_maps, list(range(NCORES)), trace=trace)
    if trace:
        kernel.last_exec_time_ns = res.exec_time_ns
        kernel.last_results = res
    out = np.concatenate(
        [res.results[c]["out"].astype(np.float32) for c in range(NCORES)], axis=0
    )
    return out


kernel.last_exec_time_ns = None
kernel.last_results = None


# revision 14
# speedup vs baseline: 1.1799x; 1.1799x over previous
"""MatchNet kernel for 8 Trainium2 NeuronCores.

Math (per batch b):
    keys   = q[b] @ W + bias
    scores = p[b] @ keys^T
    attn   = softmax(scores, axis=-1)
    out[b] = relu(attn @ q[b])

The Dense bias is dropped: softmax over lq is invariant to the per-lp
constant p@b^T it adds to scores, and keys are not used elsewhere.

Sharding: data-parallel over B=16 across 8 cores (2 batches per core).
W is broadcast. p and q are transposed on the host so every on-chip matmul
has its contraction dim on SBUF partitions.

Precision: all matmuls run as a single fp16 pass (fp32 PSUM accumulation).
Numpy sim of this exact scheme on the real seed-0 inputs: rel_err 1.67e-2
(gate 2e-2, deterministic). The previous revision added fp8-e5m2 DoubleRow
residual corrections to the two score-path matmuls (sim 9.4e-3, HW 9.85e-3,
~52us of extra PE time + 5MB more input DMA); restore those if more margin
is ever needed. Output is computed/stored fp16 (sim-identical: 1.674e-2)
and upcast to fp32 on the host, halving output DMA.
    MM1: keysT[h, lq] = sum_hk W[hk, h] * qT[hk, lq]   (fp16)
         keysT -> k16 (fp16) via DVE copy
    MM2: scores[lp, lq] = sum_h pT[h, lp] * k16[h, lq] (fp16)
    softmax over free dim; exp via ACT (bias=-rowmax, accum rowsum),
    exp output stored fp16
    T:   attnT[lq, lp] via PE transpose (fp16)
    MM3: out[lp, h] = sum_lq attnT[lq, lp] * q[lq, h]  (fp16)
    relu(out * (1/rowsum)) via ACT with per-partition scale, fp16 out
(DMA xbar transpose for attnT was tried instead of PE transposes and was
~110us SLOWER end-to-end — keep the PE-transpose path.)
"""

import os
from contextlib import ExitStack

import numpy as np

import concourse.bass as bass
import concourse.mybir as mybir
import concourse.tile as tile
from concourse import bacc
from concourse.bass import ts
from concourse.bass_utils import run_bass_kernel_spmd
from concourse.masks import make_identity

B, L, H = 16, 1024, 1024
NCORES = 8
BPC = B // NCORES  # batches per core
P = 128
KO = H // P        # 8 contraction chunks
NT = L // P        # 8 lp tiles per batch
NF = 512           # matmul moving free dim
NCH = L // NF      # 2 free chunks
F32 = mybir.dt.float32
F16 = mybir.dt.float16
AF = mybir.ActivationFunctionType
AX = mybir.AxisListType


def _build_body(ctx, tc, ins, out):
    nc = tc.nc
    pT16, qT16, qn16, W16 = ins

    const = ctx.enter_context(tc.tile_pool(name="const", bufs=1))
    ps_big = ctx.enter_context(
        tc.tile_pool(name="psbig", bufs=3, space=bass.MemorySpace.PSUM)
    )

    # PE warmup: the first ~9.5us are DMA-bound (bootstrap + first loads) and
    # the PE would sit idle, entering the kernel HAM-throttled at 1.2 GHz.
    # Zero matmuls during that window cost nothing and keep the clock-gate
    # activity window filled so K=8/8 fires soon after the real matmuls start.
    # The warmup tiles live in the long-lived pools (const / ps_big slot 0):
    # a dedicated pool whose context exits would recycle its SBUF address to
    # W16_sb[0], making the first real LDW wait on ALL warmup matmuls and
    # stalling the load descriptor queue ~2us behind pool-exit bookkeeping
    # (measured: first real MM at 16.9us instead of ~9.5us).
    wsb = const.tile([P, P], F16, name="warm_sb")
    nc.gpsimd.memset(wsb[:], 0.0)
    wps = ps_big.tile([P, L], F32, name="warm_ps", tag="ps_big")
    # The A-half (2MB) cannot land before ~14us: aggregate DMA bandwidth at
    # kernel start is only ~300-400 GB/s no matter how many queues carry it.
    # 64 pairs bridge the PE from the first possible matmul (~8us) to that
    # point with zero idle (cold ~107ns/pair until the HAM un-throttles
    # mid-warmup, ~56ns after), so the dense phase starts warm at 2.4GHz.
    # (32 pairs left a 2.4us gap: the HAM window reset and the entire first
    # m-sweep ran at 1.2GHz until 20.3us — measured ~3us slower.)
    for _ in range(32):
        nc.tensor.matmul(wps[:, :P], wsb[:], wsb[:], start=True, stop=True)

    # W tiles, one per k-chunk PAIR: chunk-pair-granular deps let the first
    # matmuls start as soon as pair 0 lands, while keeping the phase-1 b=0
    # stream at 8 descriptors total (4 sync + 4 scalar) — within the
    # framework's 8 rotating DMA semaphores, so no descriptor ever waits on
    # consumer progress during the DMA-bound start. (Per-chunk descriptors
    # measured ~2.5us slower: the 5th+ load on each queue chains to the PE.)
    W16_sb = [const.tile([P, 2, H], F16, name=f"W16_sb_{kp}") for kp in range(KO // 2)]
    ident = const.tile([P, P], F16)
    make_identity(nc, ident[:])

    qT_pool = ctx.enter_context(tc.tile_pool(name="qTp", bufs=1))
    q_pool = ctx.enter_context(tc.tile_pool(name="qp", bufs=1))
    keysT_pool = ctx.enter_context(tc.tile_pool(name="keysTp", bufs=1))
    pT_pool = ctx.enter_context(tc.tile_pool(name="pTp", bufs=3))
    attn_pool = ctx.enter_context(tc.tile_pool(name="attnp", bufs=2))
    attnT_pool = ctx.enter_context(tc.tile_pool(name="attnTp", bufs=2))
    osb_pool = ctx.enter_context(tc.tile_pool(name="osbp", bufs=2))
    stat_pool = ctx.enter_context(tc.tile_pool(name="statp", bufs=8))
    ps_t = ctx.enter_context(
        tc.tile_pool(name="pst", bufs=2, space=bass.MemorySpace.PSUM)
    )

    W16_re = W16.rearrange("(kp two ki) h -> ki kp two h", ki=P, two=2)

    for b in range(BPC):
        # qT tiles first (MM1 needs them). Issue order = consumption order
        # of the in-order PE stream.
        qT16_sb = [
            qT_pool.tile([P, 2, L], F16, name=f"qT16_sb_{b}_{kp}", tag=f"qT16_sb{kp}")
            for kp in range(KO // 2)
        ]
        qT16_re = qT16[b].rearrange("(kp two ki) l -> ki kp two l", ki=P, two=2)
        # b=0 loads run two queues in parallel (W16 on sync, qT16 on scalar —
        # scalar is idle until its first exp at ~45us) so the A-half chunks
        # land ~3us sooner during the DMA-bound kernel start. b=1's qT16 goes
        # on sync: by then scalar is busy with softmax ACTs, and spreading
        # mid-kernel loads over busy engines measured ~11us slower.
        for kp in range(KO // 2):
            if b == 0:
                nc.sync.dma_start(W16_sb[kp][:], W16_re[:, kp, :, :])
                nc.scalar.dma_start(qT16_sb[kp][:], qT16_re[:, kp, :, :])
            else:
                nc.sync.dma_start(qT16_sb[kp][:], qT16_re[:, kp, :, :])

        # ---- phase 1: keysT[h, lq] = (q @ W)^T, fp16.
        k16_sb = keysT_pool.tile([P, KO, L], F16, name=f"k16_{b}", tag="k16")
        # Each m-group is split into half-contraction sub-groups A (chunks
        # 0-3) and B (chunks 4-7), issued A0 A1 A2 B0 A3 B1 ... so the PE has
        # ~21us of A-work gated only on the first half of the 4MB phase-1
        # stream (the kernel start is DMA-bandwidth-bound). Max 3 PSUM tiles
        # live (m..m+2) matches ps_big bufs=3.
        ps_ks = {}

        def phase1_half(m, half):
            if half == 0:
                ps_ks[m] = ps_big.tile([P, L], F32, name=f"ps_k_{b}_{m}",
                                       tag="ps_big")
            ps_k = ps_ks[m]
            for n in range(NCH):
                for k in range(4 * half, 4 * half + 4):
                    nc.tensor.matmul(
                        ps_k[:, ts(n, NF)],
                        W16_sb[k // 2][:, k % 2, ts(m, P)],
                        qT16_sb[k // 2][:, k % 2, ts(n, NF)],
                        start=(k == 0),
                        stop=(k == KO - 1),
                    )
            if half == 1:
                ps_k = ps_ks.pop(m)
                nc.vector.tensor_copy(k16_sb[:, m, :], ps_k[:])

        for step in range(KO + 3):
            if step >= 3:
                phase1_half(step - 3, 1)
            if step < KO:
                phase1_half(step, 0)

        # q natural (fp16, for MM3): issued after phase-1 compute so its DMA
        # queues drain behind the phase-1-critical loads.
        qn_sb = q_pool.tile([P, KO, H], F16, name=f"qn_sb_{b}", tag="qn_sb")
        qre = qn16[b].rearrange("(kp two ki) h -> ki kp two h", ki=P, two=2)
        for kp in range(KO // 2):
            nc.sync.dma_start(qn_sb[:, 2 * kp : 2 * kp + 2, :], qre[:, kp, :, :])

        # ---- phase 2/3: per lp tile, software-pipelined
        pT16_r = pT16[b].rearrange("(ko ki) l -> ki ko l", ki=P)
        scores_ps = {}
        soft = {}

        def stage_scores(i, b=b, pT16_r=pT16_r, k16_sb=k16_sb):
            p16_sb = pT_pool.tile([P, KO, P], F16, name=f"p16_sb_{b}_{i}",
                                  tag="p16_sb")
            nc.sync.dma_start(p16_sb[:], pT16_r[:, :, ts(i, P)])
            ps_s = ps_big.tile([P, L], F32, name=f"ps_s_{b}_{i}", tag="ps_big")
            for n in range(NCH):
                for k in range(KO):
                    nc.tensor.matmul(
                        ps_s[:, ts(n, NF)],
                        p16_sb[:, k, :],
                        k16_sb[:, k, ts(n, NF)],
                        start=(k == 0),
                        stop=(k == KO - 1),
                    )
            scores_ps[i] = ps_s

        def stage_softmax_t(i, b=b):
            ps_s = scores_ps.pop(i)
            negmax = stat_pool.tile([P, 1], F32, name=f"negmax_{b}_{i}", tag="negmax")
            nc.vector.reduce_max(negmax[:], ps_s[:], axis=AX.X, negate=True)
            attn_sb = attn_pool.tile([P, L], F16, name=f"attn_{b}_{i}", tag="attn")
            rowsum = stat_pool.tile([P, 1], F32, name=f"rowsum_{b}_{i}", tag="rowsum")
            nc.scalar.activation(
                attn_sb[:],
                ps_s[:],
                AF.Exp,
                bias=negmax[:],
                accum_out=rowsum[:],
            )
            recip = stat_pool.tile([P, 1], F32, name=f"recip_{b}_{i}", tag="recip")
            nc.vector.reciprocal(recip[:], rowsum[:])

            attnT_sb = attnT_pool.tile([P, L], F16, name=f"attnT_{b}_{i}", tag="attnT")
            for g in range(L // NF):
                ps_tt = ps_t.tile([P, NF], F16, name=f"ps_tt_{b}_{i}_{g}", tag="ps_t")
                for j in range(NF // P):
                    c = g * (NF // P) + j
                    nc.tensor.transpose(
                        ps_tt[:, ts(j, P)], attn_sb[:, ts(c, P)], ident[:]
                    )
                nc.vector.tensor_copy(attnT_sb[:, ts(g, NF)], ps_tt[:])
            soft[i] = (attnT_sb, recip)

        def stage_mm3(i, b=b, qn_sb=qn_sb):
            attnT_sb, recip = soft.pop(i)
            out_sb = osb_pool.tile([P, H], F16, name=f"out_sb_{b}_{i}", tag="out_sb")
            ps_o = ps_big.tile([P, H], F32, name=f"ps_o_{b}_{i}", tag="ps_big")
            # relu+store per n-chunk so the drain of chunk 0 hides under the
            # matmuls of chunk 1 (shrinks the kernel tail). (Draining the
            # last tile in 256-col pieces was measured 0.5us SLOWER — the
            # ACT fixed cost makes two serialized half-relus beat one full.)
            for n in range(NCH):
                for k in range(KO):
                    nc.tensor.matmul(
                        ps_o[:, ts(n, NF)],
                        attnT_sb[:, ts(k, P)],
                        qn_sb[:, k, ts(n, NF)],
                        start=(k == 0),
                        stop=(k == KO - 1),
                    )
                nc.scalar.activation(
                    out_sb[:, ts(n, NF)], ps_o[:, ts(n, NF)], AF.Relu, scale=recip[:]
                )
                nc.sync.dma_start(out[b, ts(i, P), ts(n, NF)], out_sb[:, ts(n, NF)])

        stage_scores(0)
        stage_scores(1)
        for i in range(NT):
            stage_softmax_t(i)
            if i + 2 < NT:
                stage_scores(i + 2)
            stage_mm3(i)


_IN_NAMES = ["pT16", "qT16", "qn16", "W16"]

_CACHED = None


def _get_program():
    global _CACHED
    if _CACHED is not None:
        return _CACHED
    nc = bacc.Bacc(
        "TRN2",
        target_bir_lowering=False,
        debug=False,
        num_devices=NCORES,
    )
    specs = {
        "pT16": ([BPC, H, L], F16),
        "qT16": ([BPC, H, L], F16),
        "qn16": ([BPC, L, H], F16),
        "W16": ([H, H], F16),
    }
    handles = [
        nc.dram_tensor(name, *specs[name], kind="ExternalInput") for name in _IN_NAMES
    ]
    out_h = nc.dram_tensor("out", [BPC, L, H], F16, kind="ExternalOutput")
    with tile.TileContext(nc) as tc:
        with ExitStack() as ctx:
            _build_body(ctx, tc, [h.ap() for h in handles], out_h.ap())
    nc.compile()
    _CACHED = nc
    return nc


def kernel(p, q, W_key, b_key):
    # b_key is mathematically irrelevant: softmax over lq is invariant to the
    # per-lp constant p@b^T it adds to scores, and keys are not used elsewhere.
    del b_key
    p = np.ascontiguousarray(np.asarray(p, dtype=np.float32))
    q = np.ascontiguousarray(np.asarray(q, dtype=np.float32))
    W = np.ascontiguousarray(np.asarray(W_key, dtype=np.float32))

    pT16 = np.ascontiguousarray(p.transpose(0, 2, 1)).astype(np.float16)
    qT16 = np.ascontiguousarray(q.transpose(0, 2, 1)).astype(np.float16)
    qn16 = q.astype(np.float16)
    W16 = W.astype(np.float16)

    full = {"pT16": pT16, "qT16": qT16, "qn16": qn16}

    in_maps = []
    for c in range(NCORES):
        sl = slice(c * BPC, (c + 1) * BPC)
        m = {k: np.ascontiguousarray(v[sl]) for k, v in full.items()}
        m["W16"] = W16
        in_maps.append(m)

    nc = _get_program()
    trace = bool(int(os.environ.get("MATCHNET_TRACE", "0")))
    res = run_bass_kernel_spmd(nc, in_maps, list(range(NCORES)), trace=trace)
    if trace:
        kernel.last_exec_time_ns = res.exec_time_ns
        kernel.last_results = res
    out = np.concatenate(
        [res.results[c]["out"].astype(np.float32) for c in range(NCORES)], axis=0
    )
    return out


kernel.last_exec_time_ns = None
kernel.last_results = None
